# revision 1
# baseline (speedup 1.0000x reference)
"""Trainium2 Bass kernel for the min-sum BP LDPC decoder (nn_MinSumBPDecoder).

Sharding: 8 NeuronCores = 4 batch shards (16 rows each) x 2 check halves.
Each core keeps all N vars; check-to-var and var-to-check message routing is
done on-chip with gpsimd ap_gather (per-16-partition-group index lists,
precomputed on host) plus an affine block-transpose DMA through DRAM.
The per-variable totals are pair-allreduced as full var-slot planes so the
floating-point summation order matches the reference bit-exactly.

Self-contained: host-side preprocessing of the graph tables happens inside
kernel() from the passed inputs.
"""

from contextlib import ExitStack

import numpy as np

import concourse.bass as bass
import concourse.bacc as bacc
import concourse.mybir as mybir
import concourse.tile as tile
from concourse.bass_utils import run_bass_kernel_spmd

F32 = mybir.dt.float32
I32 = mybir.dt.int32
I16 = mybir.dt.int16
Alu = mybir.AluOpType

M, N, B = 8192, 16384, 64
NCHUNK = 8
MC = M // 2           # checks per core
CPC = MC // NCHUNK    # checks per chunk (512)
VPC = N // NCHUNK     # vars per chunk (2048)
CFREE = 8 * CPC       # 4096
VFREE = 4 * VPC       # 8192
MAX_ITER, ALPHA, CLAMP, PAD_BIG = 12, 0.8, 20.0, 1.0e6


def _wrap_idx(lists, dtype=np.int16):
    n = len(lists[0])
    tab = np.zeros((128, n // 16), dtype)
    for g, l in enumerate(lists):
        tab[16 * g:16 * (g + 1), :] = np.asarray(l).reshape(n // 16, 16).T
    return tab


def _preprocess(check_adj, check_adj_mask, var_adj, var_adj_mask, L1_force=None):
    check_adj = np.asarray(check_adj)
    check_mask = np.asarray(check_adj_mask) > 0.5
    var_adj = np.asarray(var_adj)
    var_mask = np.asarray(var_adj_mask) > 0.5
    E = int(check_mask.sum())
    max_cd = check_adj.shape[1]
    max_vd = var_adj.shape[1]
    assert max_cd == 8 and max_vd <= 4
    assert check_adj.shape[0] == M and var_adj.shape[0] == N

    e_check = np.full(E, -1, np.int64)
    e_k = np.full(E, -1, np.int64)
    for c in range(M):
        for k in range(max_cd):
            if check_mask[c, k]:
                e = check_adj[c, k]
                e_check[e] = c
                e_k[e] = k
    e_var = np.full(E, -1, np.int64)
    e_jglob = np.full(E, -1, np.int64)
    for v in range(N):
        for j in range(max_vd):
            if var_mask[v, j]:
                e = var_adj[v, j]
                e_var[e] = v
                e_jglob[e] = j
    assert (e_check >= 0).all() and (e_var >= 0).all()

    halves = []
    for hh in range(2):
        c_lo, c_hi = hh * MC, (hh + 1) * MC
        edges = np.nonzero((e_check >= c_lo) & (e_check < c_hi))[0]
        cr = e_check[edges] - c_lo
        u = cr // CPC
        cc = cr % CPC
        k = e_k[edges]
        v = e_var[edges]
        q = v // VPC
        vv = v % VPC
        jg = e_jglob[edges]

        run_key = u * NCHUNK + q
        rank = np.zeros(len(edges), np.int64)
        run_len = np.zeros((NCHUNK, NCHUNK), np.int64)
        order2 = np.lexsort((jg, vv, run_key))
        rk, prev = 0, -1
        for idx in order2:
            if run_key[idx] != prev:
                rk, prev = 0, run_key[idx]
            rank[idx] = rk
            rk += 1
        for i in range(len(edges)):
            run_len[u[i], q[i]] += 1

        L1 = int(run_len.max()) + 1
        L1 = (L1 + 3) & ~3
        if L1_force is not None:
            assert L1_force >= L1
            L1 = L1_force
        SFREE = NCHUNK * L1

        ZC = CFREE                     # zero cell in check planes
        preC = np.full((NCHUNK, SFREE), ZC, np.int64)
        zero_staged = L1 - 1           # run (u=0, q) last slot is always a pad
        postV = np.full((NCHUNK, VFREE), zero_staged, np.int64)
        BC = VFREE                     # big cell in var planes
        preV = np.full((NCHUNK, SFREE), BC, np.int64)
        BS = SFREE                     # big cell in staged2
        postC = np.full((NCHUNK, CFREE), BS, np.int64)

        for i in range(len(edges)):
            ui, qi, ri = u[i], q[i], rank[i]
            cpos = k[i] * CPC + cc[i]
            vpos = jg[i] * VPC + vv[i]
            preC[ui, qi * L1 + ri] = cpos
            postV[qi, vpos] = ui * L1 + ri
            preV[qi, ui * L1 + ri] = vpos
            postC[ui, cpos] = qi * L1 + ri

        halves.append(dict(
            L1=L1, SFREE=SFREE,
            preC=_wrap_idx(list(preC)), postV=_wrap_idx(list(postV)),
            preV=_wrap_idx(list(preV)), postC=_wrap_idx(list(postC)),
        ))
    return halves


def _core_inputs(syndrome, channel_llr, halves):
    syndrome = np.asarray(syndrome)
    channel_llr = np.asarray(channel_llr)
    outs = []
    for c in range(8):
        s, hh = c >> 1, c & 1
        tabs = halves[hh]
        tsl = slice(16 * s, 16 * (s + 1))
        syn_l = syndrome[tsl, hh * MC:(hh + 1) * MC].reshape(16, NCHUNK, CPC)
        # alpha * 2^9 folded scale for the half-sign product trick
        s_alpha = (ALPHA * 512.0) * (1.0 - 2.0 * syn_l)
        s_alpha = s_alpha.transpose(1, 0, 2).reshape(128, CPC)
        llr_l = channel_llr[tsl].reshape(16, NCHUNK, VPC)
        llr_l = llr_l.transpose(1, 0, 2).reshape(128, VPC)
        synp = syn_l.transpose(1, 0, 2).reshape(128, CPC).astype(np.int32)
        outs.append({
            "s_alpha": np.ascontiguousarray(s_alpha, np.float32),
            "llr_l": np.ascontiguousarray(llr_l, np.float32),
            "synp": np.ascontiguousarray(synp),
            "preC": tabs["preC"], "postV": tabs["postV"],
            "preV": tabs["preV"], "postC": tabs["postC"],
        })
    return outs


def _build(L1, max_iter=MAX_ITER):
    SFREE = NCHUNK * L1
    nc = bacc.Bacc("TRN2", target_bir_lowering=False, debug=False, num_devices=8)

    d_salpha = nc.dram_tensor("s_alpha", [128, CPC], F32, kind="ExternalInput")
    d_llr = nc.dram_tensor("llr_l", [128, VPC], F32, kind="ExternalInput")
    d_synp = nc.dram_tensor("synp", [128, CPC], I32, kind="ExternalInput")
    d_preC = nc.dram_tensor("preC", [128, SFREE // 16], I16, kind="ExternalInput")
    d_postV = nc.dram_tensor("postV", [128, VFREE // 16], I16, kind="ExternalInput")
    d_preV = nc.dram_tensor("preV", [128, SFREE // 16], I16, kind="ExternalInput")
    d_postC = nc.dram_tensor("postC", [128, CFREE // 16], I16, kind="ExternalInput")
    d_marg = nc.dram_tensor("marg", [128, VPC], F32, kind="ExternalOutput")
    d_hard = nc.dram_tensor("hard", [128, VPC], I32, kind="ExternalOutput")
    d_mism = nc.dram_tensor("mism", [128, 1], F32, kind="ExternalOutput")
    d_ccin = nc.dram_tensor("ccin", [128, VFREE], F32)
    d_ccout = nc.dram_tensor("ccout", [128, VFREE], F32)
    d_srt = nc.dram_tensor("srt", [128, SFREE], F32)

    with tile.TileContext(nc) as tc:
        with ExitStack() as ctx:
            cpool = ctx.enter_context(tc.tile_pool(name="consts", bufs=1))
            wpool = ctx.enter_context(tc.tile_pool(name="work", bufs=1))

            salpha = cpool.tile([128, CPC], F32, tag="salpha")
            llr = cpool.tile([128, VPC], F32, tag="llr")
            synp = cpool.tile([128, CPC], I32, tag="synp")
            preC = cpool.tile([128, SFREE // 16], I16, tag="preC")
            postV = cpool.tile([128, VFREE // 16], I16, tag="postV")
            preV = cpool.tile([128, SFREE // 16], I16, tag="preV")
            postC = cpool.tile([128, CFREE // 16], I16, tag="postC")
            for t, d in ((salpha, d_salpha), (llr, d_llr), (synp, d_synp),
                         (preC, d_preC), (postV, d_postV), (preV, d_preV),
                         (postC, d_postC)):
                nc.sync.dma_start(t[:], d.ap()[:])

            ctv_c = wpool.tile([128, CFREE + 4], F32, tag="ctv_c")
            vtc_c = wpool.tile([128, CFREE], F32, tag="vtc_c")
            sorted_t = wpool.tile([128, SFREE], F32, tag="sorted")
            staged = wpool.tile([128, SFREE + 4], F32, tag="staged")
            planes = wpool.tile([128, VFREE + 4], F32, tag="planes")
            absb = wpool.tile([128, CFREE], F32, tag="absb")
            sgb = wpool.tile([128, CFREE], F32, tag="sgb")
            pbuf = wpool.tile([128, 6 * CPC], F32, tag="pbuf")
            sbuf2 = wpool.tile([128, 6 * CPC], F32, tag="sbuf2")
            vtfull = wpool.tile([128, VPC], F32, tag="vtfull")

            nc.vector.memset(ctv_c[:, CFREE:], 0.0)
            nc.vector.memset(staged[:, SFREE:], PAD_BIG)
            nc.vector.memset(planes[:, VFREE:], PAD_BIG)

            pitch_staged = staged.ap[0][0]

            def a2a():
                # block transpose through DRAM (canonical APs only)
                nc.sync.dma_start(d_srt.ap()[:], sorted_t[:])
                for g in range(NCHUNK):
                    dst = bass.AP(staged.tensor, 16 * g * pitch_staged,
                                  [[pitch_staged, 16], [1, SFREE]])
                    src = bass.AP(d_srt, g * L1,
                                  [[SFREE, 16], [16 * SFREE, NCHUNK], [1, L1]])
                    nc.sync.dma_start(dst, src)

            def cpl(t, k):
                return t[:, k * CPC:(k + 1) * CPC]

            def vpl(t, j):
                return t[:, j * VPC:(j + 1) * VPC]

            def route_c2v_and_vt():
                nc.gpsimd.ap_gather(sorted_t[:], ctv_c[:], preC[:],
                                    channels=128, num_elems=CFREE + 4, d=1,
                                    num_idxs=SFREE)
                a2a()
                nc.gpsimd.ap_gather(planes[:, :VFREE], staged[:], postV[:],
                                    channels=128, num_elems=SFREE + 4, d=1,
                                    num_idxs=VFREE)
                nc.sync.dma_start(d_ccin.ap()[:], planes[:, :VFREE])
                nc.gpsimd.collective_compute(
                    "AllReduce", Alu.add,
                    replica_groups=[[0, 1], [2, 3], [4, 5], [6, 7]],
                    ins=[d_ccin.ap()[:]], outs=[d_ccout.ap()[:]])
                nc.sync.dma_start(planes[:, :VFREE], d_ccout.ap()[:])
                # reference order: ((p0 + p1) + p2) + p3
                nc.vector.tensor_tensor(vtfull[:], vpl(planes, 0), vpl(planes, 1), Alu.add)
                nc.vector.tensor_tensor(vtfull[:], vtfull[:], vpl(planes, 2), Alu.add)
                nc.vector.tensor_tensor(vtfull[:], vtfull[:], vpl(planes, 3), Alu.add)

            def vtc_from_lt():
                nc.vector.tensor_tensor(vtfull[:], vtfull[:], llr[:], Alu.add)
                for j in range(4):
                    nc.vector.scalar_tensor_tensor(
                        vpl(planes, j), vpl(planes, j), -1.0, vtfull[:],
                        Alu.mult, Alu.add)
                nc.vector.tensor_scalar(planes[:, :VFREE], planes[:, :VFREE],
                                        -CLAMP, CLAMP, Alu.max, Alu.min)

            def route_v2c():
                nc.gpsimd.ap_gather(sorted_t[:], planes[:], preV[:],
                                    channels=128, num_elems=VFREE + 4, d=1,
                                    num_idxs=SFREE)
                a2a()
                nc.gpsimd.ap_gather(vtc_c[:], staged[:], postC[:],
                                    channels=128, num_elems=SFREE + 4, d=1,
                                    num_idxs=CFREE)

            def check_phase():
                x = vtc_c
                nc.scalar.activation(absb[:], x[:], mybir.ActivationFunctionType.Abs)
                nc.vector.tensor_scalar(sgb[:], x[:], 0.0, 0.5, Alu.is_ge, Alu.subtract)
                # prefix mins: pbuf[j] = P_{j+1}
                nc.vector.tensor_tensor(pbuf[:, 0:CPC], cpl(absb, 0), cpl(absb, 1), Alu.min)
                for k in range(2, 7):
                    nc.vector.tensor_tensor(
                        pbuf[:, (k - 1) * CPC:k * CPC],
                        pbuf[:, (k - 2) * CPC:(k - 1) * CPC], cpl(absb, k), Alu.min)
                # suffix mins: sbuf2[j] = S_{j+1}
                nc.vector.tensor_tensor(sbuf2[:, 5 * CPC:6 * CPC],
                                        cpl(absb, 6), cpl(absb, 7), Alu.min)
                for k in range(5, 0, -1):
                    nc.vector.tensor_tensor(
                        sbuf2[:, (k - 1) * CPC:k * CPC],
                        sbuf2[:, k * CPC:(k + 1) * CPC], cpl(absb, k), Alu.min)
                # exclusive mins: excl[k] = min(P_{k-1}, S_{k+1}) -> absb plane k
                for k in range(1, 7):
                    p_prev = cpl(absb, 0) if k == 1 else pbuf[:, (k - 2) * CPC:(k - 1) * CPC]
                    s_next = cpl(absb, 7) if k == 6 else sbuf2[:, k * CPC:(k + 1) * CPC]
                    nc.vector.tensor_tensor(cpl(absb, k), p_prev, s_next, Alu.min)
                excl = [sbuf2[:, 0:CPC]] + [cpl(absb, k) for k in range(1, 7)] \
                    + [pbuf[:, 5 * CPC:6 * CPC]]
                m = x  # vtc_c planes as product scratch (x fully consumed)
                nc.vector.tensor_tensor(cpl(m, 0), cpl(sgb, 0), cpl(sgb, 1), Alu.mult)
                nc.vector.tensor_tensor(cpl(m, 1), cpl(sgb, 2), cpl(sgb, 3), Alu.mult)
                nc.vector.tensor_tensor(cpl(m, 2), cpl(sgb, 4), cpl(sgb, 5), Alu.mult)
                nc.vector.tensor_tensor(cpl(m, 3), cpl(sgb, 6), cpl(sgb, 7), Alu.mult)
                nc.vector.tensor_tensor(cpl(m, 0), cpl(m, 0), cpl(m, 1), Alu.mult)
                nc.vector.tensor_tensor(cpl(m, 2), cpl(m, 2), cpl(m, 3), Alu.mult)
                nc.vector.tensor_tensor(cpl(m, 0), cpl(m, 0), cpl(m, 2), Alu.mult)
                tot = cpl(m, 4)
                nc.vector.tensor_tensor(tot, cpl(m, 0), salpha[:], Alu.mult)
                for k in range(8):
                    nc.vector.tensor_tensor(cpl(sgb, k), tot, cpl(sgb, k), Alu.mult)
                    nc.vector.tensor_tensor(cpl(ctv_c, k), cpl(sgb, k), excl[k], Alu.mult)

            for it in range(max_iter):
                if it == 0:
                    for j in range(4):
                        nc.vector.tensor_scalar(vpl(planes, j), llr[:],
                                                -CLAMP, CLAMP, Alu.max, Alu.min)
                else:
                    route_c2v_and_vt()
                    vtc_from_lt()
                route_v2c()
                check_phase()

            # final readout
            route_c2v_and_vt()
            nc.vector.tensor_tensor(vtfull[:], vtfull[:], llr[:], Alu.add)
            marg = wpool.tile([128, VPC], F32, tag="marg")
            nc.scalar.activation(marg[:], vtfull[:],
                                 mybir.ActivationFunctionType.Sigmoid, scale=-1.0)
            nc.sync.dma_start(d_marg.ap()[:], marg[:])
            hardf = wpool.tile([128, VPC], F32, tag="hardf")
            nc.vector.tensor_scalar(hardf[:], vtfull[:], 0.0, None, Alu.is_lt)
            hardi = wpool.tile([128, VPC], I32, tag="hardi")
            nc.vector.tensor_copy(hardi[:], hardf[:])
            nc.sync.dma_start(d_hard.ap()[:], hardi[:])

            for j in range(4):
                nc.vector.tensor_copy(vpl(planes, j), hardf[:])
            nc.gpsimd.ap_gather(sorted_t[:], planes[:], preV[:],
                                channels=128, num_elems=VFREE + 4, d=1,
                                num_idxs=SFREE)
            a2a()
            nc.vector.memset(staged[:, SFREE:], 0.0)
            nc.gpsimd.ap_gather(vtc_c[:], staged[:], postC[:],
                                channels=128, num_elems=SFREE + 4, d=1,
                                num_idxs=CFREE)
            ssum = cpl(absb, 0)
            nc.vector.tensor_tensor(ssum, cpl(vtc_c, 0), cpl(vtc_c, 1), Alu.add)
            for k in range(2, 8):
                nc.vector.tensor_tensor(ssum, ssum, cpl(vtc_c, k), Alu.add)
            ssi = wpool.tile([128, CPC], I32, tag="ssi")
            nc.vector.tensor_copy(ssi[:], ssum)
            nc.vector.tensor_scalar(ssi[:], ssi[:], 1, None, Alu.bitwise_and)
            nc.vector.tensor_tensor(ssi[:], ssi[:], synp[:], Alu.bitwise_xor)
            ssf = cpl(absb, 1)
            nc.vector.tensor_copy(ssf, ssi[:])
            mism = wpool.tile([128, 1], F32, tag="mism")
            nc.vector.tensor_reduce(mism[:], ssf, mybir.AxisListType.X, Alu.add)
            nc.sync.dma_start(d_mism.ap()[:], mism[:])

    nc.compile()
    return nc


_CACHE = {}


def kernel(syndrome, channel_llr, check_idx, var_idx, check_adj,
           check_adj_mask, var_adj, var_adj_mask, inv_perm):
    syndrome = np.asarray(syndrome)
    channel_llr = np.asarray(channel_llr)
    assert syndrome.shape == (B, M) and channel_llr.shape == (B, N)

    halves = _preprocess(check_adj, check_adj_mask, var_adj, var_adj_mask)
    Lmax = max(h["L1"] for h in halves)
    halves = _preprocess(check_adj, check_adj_mask, var_adj, var_adj_mask,
                         L1_force=Lmax)
    cores = _core_inputs(syndrome, channel_llr, halves)

    if Lmax not in _CACHE:
        _CACHE[Lmax] = _build(Lmax)
    nc = _CACHE[Lmax]

    res = run_bass_kernel_spmd(nc, cores, list(range(8))).results

    marg = np.zeros((B, N), np.float32)
    hard = np.zeros((B, N), np.int32)
    conv = np.zeros(B, bool)
    for s in range(4):
        m, h = res[2 * s]["marg"], res[2 * s]["hard"]
        marg[16*s:16*(s+1)] = m.reshape(NCHUNK, 16, VPC).transpose(1, 0, 2).reshape(16, N)
        hard[16*s:16*(s+1)] = h.reshape(NCHUNK, 16, VPC).transpose(1, 0, 2).reshape(16, N)
        mis = (res[2*s]["mism"] + res[2*s+1]["mism"]).reshape(NCHUNK, 16).sum(axis=0)
        conv[16*s:16*(s+1)] = mis == 0
    return marg, hard, conv
